# revision 4
# baseline (speedup 1.0000x reference)
"""Kendall distance kernel for Trainium2, SPMD over 8 NeuronCores.

Math: for X (B=64, T=256, N=64),
  C[i,j] = sum_{a,b,t} sign(X[b,t,i]-X[a,t,i]) * sign(X[b,t,j]-X[a,t,j])
         = 2 * sum_{a<b,t} (...)          (diagonal a=b contributes 0)
  D = (1 - C/2016) * (1 - eye(N));  output = broadcast D to (B, N, N).

Device work: the Gram matrix of the +-1 sign tensor over all unordered
batch pairs (2016 pairs x 256 t = 516096 rows), sharded across 8 cores
by cyclic batch-difference class: core c handles ring-offset classes
d in {4c+1 .. 4c+4}.  Classes 1..31 cover each unordered pair exactly
once; class 32 (core 7, slot 3) covers its 32 pairs twice, so that slot
accumulates into a second PSUM bank which the host halves for core 7.

SPMD-uniform program: every core runs identical code.  The per-core
class offset is encoded in the DATA: input = [X (64 blocks) |
roll(X, -(4c+1)) extended to 67 blocks].  Slot j in 0..3 compares
blocks (64+j .. 64+j+63) against blocks (0..63) - plain contiguous
2D access patterns, class d = 4c+1+j.

Per slot: one DVE subtract (FD=8192), one ACT Sign, 128 PE matmul
tiles [128x64] Gram-accumulated in PSUM.  Host sums the per-core
partial Grams and forms D.
"""

import numpy as np

import concourse.bass as bass  # noqa: F401  (engine types registered on import)
import concourse.bacc as bacc
import concourse.tile as tile
from concourse import mybir
from concourse.bass_utils import run_bass_kernel_spmd

B, T, N = 64, 256, 64
P = 128                       # SBUF partitions; rows per matmul tile
TH = T // P                   # 2 t-half tiles per batch row
NCORES = 8
NSLOT = 4                     # difference classes per core
EXT = B + NSLOT - 1           # second-copy blocks (67)
NBLK = B + EXT                # total input blocks per core (131)
FD1 = B * TH * N              # free elems of one 64-block span (8192)
PAIRS_HALF = 1008.0           # C_half / 1008 == C_full / 2016

_CACHE = {}


def _build_nc():
    nc = bacc.Bacc(
        "TRN2",
        target_bir_lowering=False,
        debug=False,
        num_devices=NCORES,
    )
    f32 = mybir.dt.float32
    x_dram = nc.dram_tensor("x", [P, NBLK * TH * N], f32, kind="ExternalInput")
    out_dram = nc.dram_tensor("out", [2 * N, N], f32, kind="ExternalOutput")

    with tile.TileContext(nc) as tc:
        with (
            tc.tile_pool(name="xpool", bufs=1) as xpool,
            tc.tile_pool(name="dpool", bufs=2) as dpool,
            tc.tile_pool(name="spool", bufs=2) as spool,
            tc.tile_pool(name="psum", bufs=2, space="PSUM") as psum,
            tc.tile_pool(name="opool", bufs=1) as opool,
        ):
            x_sb = xpool.tile([P, NBLK * TH * N], f32)
            nchunk = 8
            csz = (NBLK * TH * N) // nchunk  # 131*128/8 not integer; chunk by ceil
            csz = ((NBLK * TH * N) + nchunk - 1) // nchunk
            pos = 0
            while pos < NBLK * TH * N:
                end = min(pos + csz, NBLK * TH * N)
                nc.sync.dma_start(x_sb[:, pos:end], x_dram[:, pos:end])
                pos = end

            c1_ps = psum.tile([N, N], f32, tag="c1")
            c2_ps = psum.tile([N, N], f32, tag="c2")
            for j in range(NSLOT):
                diff = dpool.tile([P, FD1], f32, tag="diff")
                sign = spool.tile([P, FD1], f32, tag="sign")
                in1 = x_sb[:, 0:FD1]
                in0 = x_sb[:, (B + j) * TH * N:(B + j) * TH * N + FD1]
                nc.vector.tensor_tensor(
                    diff[:, :], in0, in1, op=mybir.AluOpType.subtract
                )
                nc.scalar.activation(
                    sign[:, :], diff[:, :], mybir.ActivationFunctionType.Sign
                )
                acc = c1_ps if j < NSLOT - 1 else c2_ps
                for m in range(B * TH):
                    s_tile = sign[:, m * N:(m + 1) * N]
                    nc.tensor.matmul(
                        acc[:, :],
                        s_tile,
                        s_tile,
                        start=(m == 0 and (j == 0 or j == NSLOT - 1)),
                        stop=(m == B * TH - 1 and (j == NSLOT - 2 or j == NSLOT - 1)),
                    )

            out_sb = opool.tile([2 * N, N], f32)
            nc.vector.tensor_copy(out_sb[0:N, :], c1_ps[:, :])
            nc.vector.tensor_copy(out_sb[N:2 * N, :], c2_ps[:, :])
            nc.sync.dma_start(out_dram[:, :], out_sb[:, :])

    nc.compile()
    return nc


def _get_nc():
    if "nc" not in _CACHE:
        _CACHE["nc"] = _build_nc()
    return _CACHE["nc"]


def _to_sbuf_layout(blocks):
    """[nb, T, N] -> [128, nb*TH*N] with free index blk*TH*N + th*N + i."""
    nb = blocks.shape[0]
    return np.ascontiguousarray(
        blocks.reshape(nb, TH, P, N).transpose(2, 0, 1, 3).reshape(P, nb * TH * N)
    )


def _prep_core_input(X, c):
    r = np.roll(X, -(NSLOT * c + 1), axis=0)
    ext = np.concatenate([X, r, r[: NSLOT - 1]], axis=0)  # 131 blocks
    return _to_sbuf_layout(ext)


def kernel(**inputs) -> np.ndarray:
    X = np.asarray(inputs["inputs"], dtype=np.float32)
    nc = _get_nc()
    in_maps = [{"x": _prep_core_input(X, c)} for c in range(NCORES)]
    res = run_bass_kernel_spmd(nc, in_maps, core_ids=list(range(NCORES)))
    C_half = np.zeros((N, N), dtype=np.float32)
    for c, r in enumerate(res.results):
        C_half += r["out"][0:N]
        C_half += r["out"][N:2 * N] * (np.float32(0.5) if c == NCORES - 1 else 1.0)
    D = (1.0 - C_half / np.float32(PAIRS_HALF)) * (
        1.0 - np.eye(N, dtype=np.float32)
    )
    return np.ascontiguousarray(
        np.broadcast_to(D[None].astype(np.float32), (B, N, N))
    )


# revision 5
# speedup vs baseline: 1.5722x; 1.5722x over previous
"""Kendall distance kernel for Trainium2, SPMD over 8 NeuronCores.

Math: for X (B=64, T=256, N=64),
  C[i,j] = sum_{a,b,t} sign(X[b,t,i]-X[a,t,i]) * sign(X[b,t,j]-X[a,t,j])
         = 2 * sum_{a<b,t} (...)          (diagonal a=b contributes 0)
  D = (1 - C/2016) * (1 - eye(N));  output = broadcast D to (B, N, N).

Device work: the Gram matrix of the +-1 sign tensor over all unordered
batch pairs (2016 pairs x 256 t = 516096 rows), sharded across 8 cores
by cyclic batch-difference class: core c handles ring-offset classes
d in {4c+1 .. 4c+4}.  Classes 1..31 cover each unordered pair exactly
once; class 32 (core 7, slot 3) covers its 32 pairs twice, so that slot
accumulates into a second PSUM bank which the host halves for core 7.

SPMD-uniform program: every core runs identical code.  The per-core
class offset is encoded in the DATA: input = [X (64 blocks) |
roll(X, -(4c+1)) extended to 67 blocks].  Slot j in 0..3 compares
blocks (64+j .. 64+j+63) against blocks (0..63) - plain contiguous
2D access patterns, class d = 4c+1+j.

Per slot: one DVE subtract (FD=8192), one ACT Sign, 128 PE matmul
tiles [128x64] Gram-accumulated in PSUM.  Host sums the per-core
partial Grams and forms D.
"""

import numpy as np

import concourse.bass as bass  # noqa: F401  (engine types registered on import)
import concourse.bacc as bacc
import concourse.tile as tile
from concourse import mybir
from concourse.bass_utils import run_bass_kernel_spmd

B, T, N = 64, 256, 64
P = 128                       # SBUF partitions; rows per matmul tile
TH = T // P                   # 2 t-half tiles per batch row
NCORES = 8
NSLOT = 4                     # difference classes per core
EXT = B + NSLOT - 1           # second-copy blocks (67)
NBLK = B + EXT                # total input blocks per core (131)
FD1 = B * TH * N              # free elems of one 64-block span (8192)
PAIRS_HALF = 1008.0           # C_half / 1008 == C_full / 2016

_CACHE = {}


def _build_nc():
    nc = bacc.Bacc(
        "TRN2",
        target_bir_lowering=False,
        debug=False,
        num_devices=NCORES,
    )
    f32 = mybir.dt.float32
    bf16 = mybir.dt.bfloat16
    x_dram = nc.dram_tensor("x", [P, NBLK * TH * N], f32, kind="ExternalInput")
    out_dram = nc.dram_tensor("out", [2 * N, N], f32, kind="ExternalOutput")

    SUB = 4                       # chunks per slot (pipelining granularity)
    CB = B // SUB                 # batch-blocks per chunk (16)
    CFD = CB * TH * N             # free elems per chunk (2048)

    with tile.TileContext(nc) as tc:
        with (
            tc.tile_pool(name="xpool", bufs=1) as xpool,
            tc.tile_pool(name="dpool", bufs=3) as dpool,
            tc.tile_pool(name="spool", bufs=3) as spool,
            tc.tile_pool(name="psum", bufs=2, space="PSUM") as psum,
            tc.tile_pool(name="opool", bufs=1) as opool,
        ):
            x_sb = xpool.tile([P, NBLK * TH * N], f32)
            # DMA the second (rolled) copy first: the first compute chunk
            # needs blocks 64.. plus blocks 0..; issue in consumption order.
            bounds = [B * TH * N, NBLK * TH * N, 0]
            nchunk = 8
            spans = []
            for lo, hi in ((B * TH * N, NBLK * TH * N), (0, B * TH * N)):
                csz = (hi - lo + nchunk // 2 - 1) // (nchunk // 2)
                pos = lo
                while pos < hi:
                    end = min(pos + csz, hi)
                    spans.append((pos, end))
                    pos = end
            for lo, hi in spans:
                nc.sync.dma_start(x_sb[:, lo:hi], x_dram[:, lo:hi])

            c1_ps = psum.tile([N, N], f32, tag="c1")
            c2_ps = psum.tile([N, N], f32, tag="c2")
            n_acc1 = (NSLOT - 1) * SUB * CB * TH
            n_acc2 = SUB * CB * TH
            k1 = k2 = 0
            for j in range(NSLOT):
                acc = c1_ps if j < NSLOT - 1 else c2_ps
                for s in range(SUB):
                    diff = dpool.tile([P, CFD], f32, tag="diff")
                    sign = spool.tile([P, CFD], bf16, tag="sign")
                    base1 = s * CFD
                    base0 = (B + j) * TH * N + s * CFD
                    nc.vector.tensor_tensor(
                        diff[:, :],
                        x_sb[:, base0:base0 + CFD],
                        x_sb[:, base1:base1 + CFD],
                        op=mybir.AluOpType.subtract,
                    )
                    nc.scalar.activation(
                        sign[:, :], diff[:, :], mybir.ActivationFunctionType.Sign
                    )
                    for m in range(CB * TH):
                        s_tile = sign[:, m * N:(m + 1) * N]
                        if j < NSLOT - 1:
                            st, sp = k1 == 0, k1 == n_acc1 - 1
                            k1 += 1
                        else:
                            st, sp = k2 == 0, k2 == n_acc2 - 1
                            k2 += 1
                        nc.tensor.matmul(acc[:, :], s_tile, s_tile, start=st, stop=sp)

            out_sb = opool.tile([2 * N, N], f32)
            nc.vector.tensor_copy(out_sb[0:N, :], c1_ps[:, :])
            nc.vector.tensor_copy(out_sb[N:2 * N, :], c2_ps[:, :])
            nc.sync.dma_start(out_dram[:, :], out_sb[:, :])

    nc.compile()
    return nc


def _get_nc():
    if "nc" not in _CACHE:
        _CACHE["nc"] = _build_nc()
    return _CACHE["nc"]


def _to_sbuf_layout(blocks):
    """[nb, T, N] -> [128, nb*TH*N] with free index blk*TH*N + th*N + i."""
    nb = blocks.shape[0]
    return np.ascontiguousarray(
        blocks.reshape(nb, TH, P, N).transpose(2, 0, 1, 3).reshape(P, nb * TH * N)
    )


def _prep_core_input(X, c):
    r = np.roll(X, -(NSLOT * c + 1), axis=0)
    ext = np.concatenate([X, r, r[: NSLOT - 1]], axis=0)  # 131 blocks
    return _to_sbuf_layout(ext)


def kernel(**inputs) -> np.ndarray:
    X = np.asarray(inputs["inputs"], dtype=np.float32)
    nc = _get_nc()
    in_maps = [{"x": _prep_core_input(X, c)} for c in range(NCORES)]
    res = run_bass_kernel_spmd(nc, in_maps, core_ids=list(range(NCORES)))
    C_half = np.zeros((N, N), dtype=np.float32)
    for c, r in enumerate(res.results):
        C_half += r["out"][0:N]
        C_half += r["out"][N:2 * N] * (np.float32(0.5) if c == NCORES - 1 else 1.0)
    D = (1.0 - C_half / np.float32(PAIRS_HALF)) * (
        1.0 - np.eye(N, dtype=np.float32)
    )
    return np.ascontiguousarray(
        np.broadcast_to(D[None].astype(np.float32), (B, N, N))
    )


# revision 10
# speedup vs baseline: 2.0424x; 1.2991x over previous
"""Kendall distance kernel for Trainium2, SPMD over 8 NeuronCores.

Math: for X (B=64, T=256, N=64),
  C[i,j] = sum_{a,b,t} sign(X[b,t,i]-X[a,t,i]) * sign(X[b,t,j]-X[a,t,j])
         = 2 * sum_{a<b,t} (...)          (diagonal a=b contributes 0)
  D = (1 - C/2016) * (1 - eye(N));  output = broadcast D to (B, N, N).

Device work: the Gram matrix of the +-1 sign tensor over all unordered
batch pairs (2016 pairs x 256 t = 516096 rows), sharded across 8 cores
by cyclic batch-difference class: core c handles ring-offset classes
d in {4c+1 .. 4c+4}.  Classes 1..31 cover each unordered pair exactly
once; class 32 (core 7, slot 3) covers its 32 pairs twice, so that slot
accumulates into a second PSUM accumulator which the host halves for
core 7.

SPMD-uniform program: every core runs identical code.  The per-core
class offset is encoded in the DATA: input = [R (64 blocks) |
roll(R, -(4c+1)) extended to 67 blocks] where R is the per-(t,i)-column
RANK transform of X (exact in bf16; sign(rank diff) == sign(value diff)
except exact ties, which the host corrects - see _tie_correction).

Per chunk: DVE bf16 subtract (2x mode), ACT Sign -> bf16 +-1, PE
matmuls column-paired via tile_position so two 64-col weight loads
occupy both halves of the 128x128 array.  Host sums the per-core
partial Grams (each PSUM accumulator holds two 64-row halves) and
forms D.
"""

import numpy as np
import ml_dtypes

import concourse.bass as bass  # noqa: F401
import concourse.bacc as bacc
import concourse.tile as tile
from concourse import mybir
from concourse.bass_utils import run_bass_kernel_spmd

B, T, N = 64, 256, 64
P = 128                       # SBUF partitions; rows per matmul tile
TH = T // P                   # 2 t-half tiles per batch row
NCORES = 8
NSLOT = 4                     # difference classes per core
EXT = B + NSLOT - 1           # second-copy blocks (67)
NBLK = B + EXT                # total input blocks per core (131)
FD1 = B * TH * N              # free elems of one 64-block span (8192)
PAIRS_HALF = 1008.0

_CACHE = {}


def _build_nc():
    nc = bacc.Bacc(
        "TRN2",
        target_bir_lowering=False,
        debug=False,
        num_devices=NCORES,
    )
    f32 = mybir.dt.float32
    bf16 = mybir.dt.bfloat16
    x_dram = nc.dram_tensor("x", [P, NBLK * TH * N], bf16, kind="ExternalInput")
    out_dram = nc.dram_tensor("out", [P, 2 * N], f32, kind="ExternalOutput")

    SUB = 4                       # chunks per slot
    CB = B // SUB                 # batch-blocks per chunk (16)
    CFD = CB * TH * N             # free elems per chunk (2048)
    MT = CB * TH                  # matmul tiles per chunk (32)

    with tile.TileContext(nc) as tc:
        with (
            tc.tile_pool(name="xpool", bufs=1) as xpool,
            tc.tile_pool(name="dpool", bufs=3) as dpool,
            tc.tile_pool(name="spool", bufs=3) as spool,
            tc.tile_pool(name="psum", bufs=2, space="PSUM") as psum,
            tc.tile_pool(name="opool", bufs=1) as opool,
        ):
            x_sb = xpool.tile([P, NBLK * TH * N], bf16)
            nchunk = 8
            spans = []
            for lo, hi in ((B * TH * N, NBLK * TH * N), (0, B * TH * N)):
                csz = (hi - lo + nchunk // 2 - 1) // (nchunk // 2)
                pos = lo
                while pos < hi:
                    end = min(pos + csz, hi)
                    spans.append((pos, end))
                    pos = end
            for lo, hi in spans:
                nc.sync.dma_start(x_sb[:, lo:hi], x_dram[:, lo:hi])

            # one PSUM bank per (accumulator, column-half): the PE writes the
            # two array column-halves to different partition ranges, and the
            # group tracker wants one pending group per bank.
            acc_ps = {
                (w, h): psum.tile([P, N], f32, tag=f"c{w}{h}", name=f"c{w}{h}_ps")
                for w in (0, 1)
                for h in (0, 1)
            }
            n1 = (NSLOT - 1) * SUB * MT // 2
            n2 = SUB * MT // 2
            cnt = {(0, 0): 0, (0, 1): 0, (1, 0): 0, (1, 1): 0}
            tot = {(0, 0): n1, (0, 1): n1, (1, 0): n2, (1, 1): n2}
            for j in range(NSLOT):
                which = 0 if j < NSLOT - 1 else 1
                for s in range(SUB):
                    diff = dpool.tile([P, CFD], bf16, tag="diff")
                    sign = spool.tile([P, CFD], bf16, tag="sign")
                    base1 = s * CFD
                    base0 = (B + j) * TH * N + s * CFD
                    nc.vector.tensor_tensor(
                        diff[:, :],
                        x_sb[:, base0:base0 + CFD],
                        x_sb[:, base1:base1 + CFD],
                        op=mybir.AluOpType.subtract,
                    )
                    nc.scalar.activation(
                        sign[:, :], diff[:, :], mybir.ActivationFunctionType.Sign
                    )
                    for m in range(MT):
                        half = m & 1
                        s_tile = sign[:, m * N:(m + 1) * N]
                        k = cnt[(which, half)]
                        cnt[(which, half)] += 1
                        nc.tensor.matmul(
                            acc_ps[(which, half)][half * N:(half + 1) * N, :],
                            s_tile,
                            s_tile,
                            start=(k == 0),
                            stop=(k == tot[(which, half)] - 1),
                            tile_position=(0, half * N),
                        )

            out_sb = opool.tile([P, 2 * N], f32)
            for w in (0, 1):
                for h in (0, 1):
                    nc.vector.tensor_copy(
                        out_sb[h * N:(h + 1) * N, w * N:(w + 1) * N],
                        acc_ps[(w, h)][h * N:(h + 1) * N, :],
                    )
            nc.sync.dma_start(out_dram[:, :], out_sb[:, :])

    nc.compile()
    return nc


def _get_nc():
    if "nc" not in _CACHE:
        _CACHE["nc"] = _build_nc()
    return _CACHE["nc"]


def _ranks(X):
    """Per-(t,i)-column batch ranks, 0..B-1, exact in bf16."""
    order = np.argsort(X, axis=0, kind="stable")
    ranks = np.empty_like(order)
    np.put_along_axis(
        ranks, order, np.arange(B, dtype=order.dtype)[:, None, None], axis=0
    )
    return ranks.astype(np.float32)


def _to_sbuf_layout(blocks):
    nb = blocks.shape[0]
    return np.ascontiguousarray(
        blocks.reshape(nb, TH, P, N)
        .transpose(2, 0, 1, 3)
        .reshape(P, nb * TH * N)
        .astype(ml_dtypes.bfloat16)
    )


def _prep_core_input(R, c):
    r = np.roll(R, -(NSLOT * c + 1), axis=0)
    ext = np.concatenate([R, r, r[: NSLOT - 1]], axis=0)  # 131 blocks
    return _to_sbuf_layout(ext)


def _tie_correction(X, ranks):
    """Exact fix for within-column value ties: the rank-sign kernel counts
    sign(rank diff)=+-1 where the true sign is 0.  Subtract, for every tied
    (a,b,t) event, rank_sign_i * rank_sign_j on row/col i in tied columns."""
    C_fix = np.zeros((N, N), dtype=np.float64)
    Xs = np.sort(X, axis=0)
    t_idx, i_idx = np.nonzero((Xs[1:] == Xs[:-1]).any(axis=0))
    events = {}
    for t, i in zip(t_idx, i_idx):
        col = X[:, t, i]
        order = np.argsort(col, kind="stable")
        sc = col[order]
        for k in np.nonzero(sc[1:] == sc[:-1])[0]:
            a, b = order[k], order[k + 1]
            events.setdefault((min(a, b), max(a, b), t), []).append(i)
    for (a, b, t), cols in events.items():
        shat = np.sign(ranks[b, t, :] - ranks[a, t, :])
        W = np.outer(shat, shat)
        mask = np.zeros((N, N), dtype=bool)
        mask[cols, :] = True
        mask[:, cols] = True
        C_fix += W * mask
    return C_fix.astype(np.float32)


def kernel(**inputs) -> np.ndarray:
    X = np.asarray(inputs["inputs"], dtype=np.float32)
    R = _ranks(X)
    nc = _get_nc()
    in_maps = [{"x": _prep_core_input(R, c)} for c in range(NCORES)]
    res = run_bass_kernel_spmd(nc, in_maps, core_ids=list(range(NCORES)))
    C_half = np.zeros((N, N), dtype=np.float32)
    for c, r in enumerate(res.results):
        o = r["out"]
        C_half += o[0:N, 0:N] + o[N:P, 0:N]
        w = np.float32(0.5) if c == NCORES - 1 else np.float32(1.0)
        C_half += (o[0:N, N:2 * N] + o[N:P, N:2 * N]) * w
    C_half -= _tie_correction(X, R)
    D = (1.0 - C_half / np.float32(PAIRS_HALF)) * (
        1.0 - np.eye(N, dtype=np.float32)
    )
    return np.ascontiguousarray(
        np.broadcast_to(D[None].astype(np.float32), (B, N, N))
    )


# revision 11
# speedup vs baseline: 2.6000x; 1.2730x over previous
"""Kendall distance kernel for Trainium2, SPMD over 8 NeuronCores.

Math: for X (B=64, T=256, N=64),
  C[i,j] = sum_{a,b,t} sign(X[b,t,i]-X[a,t,i]) * sign(X[b,t,j]-X[a,t,j])
         = 2 * sum_{a<b,t} (...)          (diagonal a=b contributes 0)
  D = (1 - C/2016) * (1 - eye(N));  output = broadcast D to (B, N, N).

Device work: the Gram matrix of the +-1 sign tensor over all unordered
batch pairs (2016 pairs x 256 t = 516096 rows), sharded across 8 cores
by cyclic batch-difference class: core c handles ring-offset classes
d in {4c+1 .. 4c+4}.  Classes 1..31 cover each unordered pair exactly
once; class 32 (core 7, slot 3) covers its 32 pairs twice, so slot-3
chunks accumulate into a second PSUM accumulator which the host halves
for core 7.

SPMD-uniform program: every core runs identical code.  The per-core
class offset is encoded in the DATA: inputs are R (64 blocks, natural
order) and 4 overlapping 19-block windows of roll(R, -(4c+1)) extended,
where R is the per-(t,i)-column RANK transform of X (exact in bf16;
sign(rank diff) == sign(value diff) except exact value ties, which the
host corrects - see _tie_correction).

Per chunk (16 blocks x one class): DVE bf16 subtract (2x mode), then
either ACT Sign or a DVE min/max clamp to +-1 (4x mode) - chunks are
split between the two engines to balance their load.  PE consumes
sign tiles PAIRED: W = [S_2k | S_2k+1] (128x128 bf16) in a single
FWL-eligible matmul; the diagonal 64x64 blocks of W^T W are the two
tiles' Grams (off-diagonal blocks are discarded by the host).
"""

import numpy as np
import ml_dtypes

import concourse.bass as bass  # noqa: F401
import concourse.bacc as bacc
import concourse.tile as tile
from concourse import mybir
from concourse.bass_utils import run_bass_kernel_spmd

B, T, N = 64, 256, 64
P = 128
TH = T // P                   # 2
NCORES = 8
NSLOT = 4
SUB = 4                       # chunks per slot
CB = B // SUB                 # blocks per chunk (16)
WB = CB + NSLOT - 1           # blocks per x2 window (19)
BFD = TH * N                  # free elems per block (128)
CFD = CB * BFD                # free elems per chunk (2048)
PMT = CB * TH // 2            # paired matmuls per chunk (16)
PAIRS_HALF = 1008.0

_CACHE = {}


def _build_nc():
    nc = bacc.Bacc(
        "TRN2",
        target_bir_lowering=False,
        debug=False,
        num_devices=NCORES,
    )
    f32 = mybir.dt.float32
    bf16 = mybir.dt.bfloat16
    x1_dram = nc.dram_tensor("x1", [P, B * BFD], bf16, kind="ExternalInput")
    x2_dram = nc.dram_tensor("x2", [P, SUB * WB * BFD], bf16, kind="ExternalInput")
    out_dram = nc.dram_tensor("out", [P, 2 * P], f32, kind="ExternalOutput")

    with tile.TileContext(nc) as tc:
        with (
            tc.tile_pool(name="xpool", bufs=1) as xpool,
            tc.tile_pool(name="dpool", bufs=4) as dpool,
            tc.tile_pool(name="spool", bufs=4) as spool,
            tc.tile_pool(name="psum", bufs=2, space="PSUM") as psum,
            tc.tile_pool(name="opool", bufs=1) as opool,
        ):
            x1t = [
                xpool.tile([P, CFD], bf16, tag=f"x1t{s}", name=f"x1t{s}")
                for s in range(SUB)
            ]
            x2t = [
                xpool.tile([P, WB * BFD], bf16, tag=f"x2t{s}", name=f"x2t{s}")
                for s in range(SUB)
            ]
            for s in range(SUB):
                nc.sync.dma_start(
                    x2t[s][:, :], x2_dram[:, s * WB * BFD:(s + 1) * WB * BFD]
                )
                nc.sync.dma_start(x1t[s][:, :], x1_dram[:, s * CFD:(s + 1) * CFD])

            c1_ps = psum.tile([P, P], f32, tag="c1")
            c2_ps = psum.tile([P, P], f32, tag="c2")
            n1 = (NSLOT - 1) * SUB * PMT
            n2 = SUB * PMT
            k1 = k2 = 0
            for s in range(SUB):
                for j in range(NSLOT):
                    idx = s * NSLOT + j
                    diff = dpool.tile([P, CFD], bf16, tag="diff", name=f"diff{idx}")
                    sign = spool.tile([P, CFD], bf16, tag="sign", name=f"sign{idx}")
                    nc.vector.tensor_tensor(
                        diff[:, :],
                        x2t[s][:, j * BFD:j * BFD + CFD],
                        x1t[s][:, :],
                        op=mybir.AluOpType.subtract,
                    )
                    if idx % 3 == 1:
                        # DVE route: clamp nonzero integer diffs to +-1
                        nc.vector.tensor_scalar(
                            sign[:, :],
                            diff[:, :],
                            1.0,
                            -1.0,
                            op0=mybir.AluOpType.min,
                            op1=mybir.AluOpType.max,
                        )
                    else:
                        nc.scalar.activation(
                            sign[:, :],
                            diff[:, :],
                            mybir.ActivationFunctionType.Sign,
                        )
                    for m in range(PMT):
                        w_tile = sign[:, m * P:(m + 1) * P]
                        if j < NSLOT - 1:
                            st, sp = k1 == 0, k1 == n1 - 1
                            k1 += 1
                            acc = c1_ps
                        else:
                            st, sp = k2 == 0, k2 == n2 - 1
                            k2 += 1
                            acc = c2_ps
                        nc.tensor.matmul(
                            acc[:, :], w_tile, w_tile, start=st, stop=sp
                        )

            out_sb = opool.tile([P, 2 * P], f32)
            nc.vector.tensor_copy(out_sb[:, 0:P], c1_ps[:, :])
            nc.vector.tensor_copy(out_sb[:, P:2 * P], c2_ps[:, :])
            nc.sync.dma_start(out_dram[:, :], out_sb[:, :])

    nc.compile()
    return nc


def _get_nc():
    if "nc" not in _CACHE:
        _CACHE["nc"] = _build_nc()
    return _CACHE["nc"]


def _ranks(X):
    """Per-(t,i)-column batch ranks, 0..B-1, exact in bf16."""
    order = np.argsort(X, axis=0, kind="stable")
    ranks = np.empty_like(order)
    np.put_along_axis(
        ranks, order, np.arange(B, dtype=order.dtype)[:, None, None], axis=0
    )
    return ranks.astype(np.float32)


def _to_sbuf_layout(blocks):
    nb = blocks.shape[0]
    return np.ascontiguousarray(
        blocks.reshape(nb, TH, P, N)
        .transpose(2, 0, 1, 3)
        .reshape(P, nb * BFD)
        .astype(ml_dtypes.bfloat16)
    )


def _prep_core_inputs(R, c):
    r = np.roll(R, -(NSLOT * c + 1), axis=0)
    ext = np.concatenate([r, r[: NSLOT - 1]], axis=0)  # 67 blocks
    win = np.concatenate(
        [ext[CB * s:CB * s + WB] for s in range(SUB)], axis=0
    )  # 76 blocks
    return {"x1": _to_sbuf_layout(R), "x2": _to_sbuf_layout(win)}


def _tie_correction(X, ranks):
    """Exact fix for within-column value ties: the rank-sign kernel counts
    sign(rank diff)=+-1 where the true sign is 0."""
    C_fix = np.zeros((N, N), dtype=np.float64)
    Xs = np.sort(X, axis=0)
    t_idx, i_idx = np.nonzero((Xs[1:] == Xs[:-1]).any(axis=0))
    events = {}
    for t, i in zip(t_idx, i_idx):
        col = X[:, t, i]
        order = np.argsort(col, kind="stable")
        sc = col[order]
        for k in np.nonzero(sc[1:] == sc[:-1])[0]:
            a, b = order[k], order[k + 1]
            events.setdefault((min(a, b), max(a, b), t), []).append(i)
    for (a, b, t), cols in events.items():
        shat = np.sign(ranks[b, t, :] - ranks[a, t, :])
        W = np.outer(shat, shat)
        mask = np.zeros((N, N), dtype=bool)
        mask[cols, :] = True
        mask[:, cols] = True
        C_fix += W * mask
    return C_fix.astype(np.float32)


def kernel(**inputs) -> np.ndarray:
    X = np.asarray(inputs["inputs"], dtype=np.float32)
    R = _ranks(X)
    nc = _get_nc()
    in_maps = [_prep_core_inputs(R, c) for c in range(NCORES)]
    res = run_bass_kernel_spmd(nc, in_maps, core_ids=list(range(NCORES)))
    C_half = np.zeros((N, N), dtype=np.float32)
    for c, r in enumerate(res.results):
        o = r["out"]
        C_half += o[0:N, 0:N] + o[N:P, N:P]
        w = np.float32(0.5) if c == NCORES - 1 else np.float32(1.0)
        C_half += (o[0:N, P:P + N] + o[N:P, P + N:2 * P]) * w
    C_half -= _tie_correction(X, R)
    D = (1.0 - C_half / np.float32(PAIRS_HALF)) * (
        1.0 - np.eye(N, dtype=np.float32)
    )
    return np.ascontiguousarray(
        np.broadcast_to(D[None].astype(np.float32), (B, N, N))
    )
